# revision 4
# baseline (speedup 1.0000x reference)
"""Causal GQA self-attention (B=2, T=2048, D=2048, 16 q-heads / 4 kv-heads,
head_dim=128, full-dim RoPE) on 8 Trainium2 NeuronCores.

Strategy: tensor-parallel over heads. Core c owns q-heads {2c, 2c+1} and
kv-head c//2. Wq/Wkv output dims and Wproj input dims are sharded 8-ways on
the host; each core computes a full-width partial of the output projection
(bf16) and the host sums the 8 partials in f32.

On-chip layout: x is staged transposed (channel-major [C, B*T]) so the
QKV projections need no on-chip transpose; attention is computed "k-major"
(scores transposed, [k_pos, q_pos]) so the P@V contraction needs no
transpose either. Softmax runs without max-subtraction (scores are ~N(0,1);
exp never overflows) and the denominator comes from an all-ones stationary
matmul which also broadcasts the per-column sums across all partitions.

Scheduling notes (v2):
- RoPE runs in bf16: the projection PSUM is staged to SBUF via an ACT copy
  (scalar engine is idle in phase 1) so every DVE op is all-bf16/SBUF and
  hits the 2x perf mode. The rotate-half sign is folded into the sin table.
- Output partials are written bf16 (halves the 33MB write DMA).
- The out-projection for chunk qc-1 is interleaved into chunk qc's
  attention stream at k-tile granularity so its PSUM->SBUF copies never
  form a burst ahead of the next exp in the ACT queue.
- PSUM tags: mm(2) sc(2) y(2) s(2) = 8 banks, no cross-phase sharing.
- DMA emission order: wq/xt-chunk0 interleaved per k-tile, then cos/sin,
  wk/wv, remaining chunks, tri, wp last; batch-1 xt is emitted right after
  batch-0's QKV loop so it streams during batch-0 attention.
"""

import math
import os
import sys

for _p in ("/opt/trn_rl_repo", "/root/.axon_site/_ro/trn_rl_repo"):
    if os.path.isdir(_p) and _p not in sys.path:
        sys.path.insert(0, _p)

import ml_dtypes
import numpy as np

BF16 = ml_dtypes.bfloat16

B = 2
T = 2048
C = 2048
D = 128          # head dim
NQH = 2          # q heads per core
TOK = B * T      # 4096
KT = C // 128    # 16 contraction tiles
NCH = 512        # matmul moving-dim chunk
QCH = T // NCH   # 4 q chunks per batch
KB = T // 128    # 16 k tiles per batch
N_CORES = 8
SCALE = 1.0 / math.sqrt(D)

_COMPILED = {}


def _rope_tables():
    dim = np.arange(D // 2, dtype=np.float64)
    freq = 10000.0 ** (dim / (D / 2))
    freq = np.concatenate([freq, freq])              # [128]
    pos = np.arange(T, dtype=np.float64)
    ang = pos[None, :] / freq[:, None]               # [128, T] channel-major
    cos = np.cos(ang)
    sin_s = np.sin(ang)
    sin_s[: D // 2] = -sin_s[: D // 2]               # rotate-half sign folded
    return cos, sin_s


def _build_nc(debug=False):
    import concourse.bass as bass  # noqa: F401
    import concourse.mybir as mybir
    import concourse.tile as tile
    from concourse import bacc
    from concourse.bass import ts

    f32 = mybir.dt.float32
    bf16 = mybir.dt.bfloat16
    AF = mybir.ActivationFunctionType
    OP = mybir.AluOpType

    nc = bacc.Bacc("TRN2", target_bir_lowering=False, debug=False,
                   num_devices=N_CORES)

    xt_e = nc.dram_tensor("xt", [C, TOK], bf16, kind="ExternalInput")
    wq_e = nc.dram_tensor("wq", [C, NQH * D], bf16, kind="ExternalInput")
    wk_e = nc.dram_tensor("wk", [C, D], bf16, kind="ExternalInput")
    wv_e = nc.dram_tensor("wv", [C, D], bf16, kind="ExternalInput")
    wp_e = nc.dram_tensor("wp", [NQH * D, C], bf16, kind="ExternalInput")
    cos_e = nc.dram_tensor("cos", [D, T], bf16, kind="ExternalInput")
    sin_e = nc.dram_tensor("sin", [D, T], bf16, kind="ExternalInput")
    tri_e = nc.dram_tensor("tri", [D, D], bf16, kind="ExternalInput")
    out_e = nc.dram_tensor("out", [TOK, C], bf16, kind="ExternalOutput")

    from contextlib import ExitStack

    with tile.TileContext(nc) as tc, ExitStack() as ctx:
        const = ctx.enter_context(tc.tile_pool(name="const", bufs=1))
        qkvp = ctx.enter_context(tc.tile_pool(name="qkv", bufs=1))
        psum = ctx.enter_context(tc.tile_pool(name="ps", bufs=2, space="PSUM"))
        xtp = ctx.enter_context(tc.tile_pool(name="xt", bufs=1))
        w1p = ctx.enter_context(tc.tile_pool(name="w1", bufs=1))
        rtp = ctx.enter_context(tc.tile_pool(name="rt", bufs=2))
        stag = ctx.enter_context(tc.tile_pool(name="stg", bufs=3))
        exp_p = ctx.enter_context(tc.tile_pool(name="exp", bufs=6))
        recp = ctx.enter_context(tc.tile_pool(name="rec", bufs=2))
        outp = ctx.enter_context(tc.tile_pool(name="outs", bufs=3))

        # ---- DMA emission order: get the first q-proj matmul running as
        # early as possible (wq/xt interleaved per k-tile), then the tables
        # needed by the first rope, then the rest in consumption order.
        wq_sb = w1p.tile([128, KT, NQH * D], bf16, tag="wq")
        xt0_sb = xtp.tile([128, KT, T], bf16, tag="xt")
        for kt in range(KT):
            nc.sync.dma_start(wq_sb[:, kt, :],
                              wq_e.ap()[kt * 128:(kt + 1) * 128, :])
            nc.sync.dma_start(xt0_sb[:, kt, 0:NCH],
                              xt_e.ap()[kt * 128:(kt + 1) * 128, 0:NCH])
        cos_sb = const.tile([D, T], bf16, tag="cos")
        nc.sync.dma_start(cos_sb[:], cos_e.ap())
        sin_sb = const.tile([D, T], bf16, tag="sin")
        nc.sync.dma_start(sin_sb[:], sin_e.ap())
        wk_sb = w1p.tile([128, KT, D], bf16, tag="wk")
        nc.sync.dma_start(wk_sb[:], wk_e.ap().rearrange("(ko p) n -> p ko n", p=128))
        wv_sb = w1p.tile([128, KT, D], bf16, tag="wv")
        nc.sync.dma_start(wv_sb[:], wv_e.ap().rearrange("(ko p) n -> p ko n", p=128))
        for tc_ in range(1, QCH):
            for kt in range(KT):
                nc.sync.dma_start(
                    xt0_sb[:, kt, ts(tc_, NCH)],
                    xt_e.ap()[kt * 128:(kt + 1) * 128,
                              tc_ * NCH:(tc_ + 1) * NCH])
        tri_sb = const.tile([D, D], bf16, tag="tri")
        nc.sync.dma_start(tri_sb[:], tri_e.ap())
        ones_sb = const.tile([128, 128], bf16, tag="ones")
        nc.vector.memset(ones_sb[:], 1.0)
        wp_sb = const.tile([128, NQH, C], bf16, tag="wp")
        nc.sync.dma_start(wp_sb[:], wp_e.ap().rearrange("(ko p) n -> p ko n", p=128))

        # persistent per-batch-pair tensors
        qT = qkvp.tile([D, NQH, TOK], bf16, tag="qT")    # rope'd, pre-scaled
        kT = qkvp.tile([D, TOK], bf16, tag="kT")         # rope'd
        vv = qkvp.tile([128, B * KB, D], bf16, tag="vv")  # token-major
        yT = qkvp.tile([D, NQH, TOK], bf16, tag="yT")    # attn out, normalized

        def rope_out(dst, src_ps, cos_ap, sin_ap):
            """dst(bf16) = src * cos + rotate_half(src) * sin_signed.

            src is staged PSUM->SBUF bf16 on the scalar engine so all DVE
            ops below run in the 2x/4x (all-bf16, all-SBUF) perf modes.
            tensor_tensor operands must share a start partition, so the
            rotate-half is done with two (partition-shift-capable) copies."""
            qbf = stag.tile([128, NCH], bf16, tag="qbf")
            nc.scalar.copy(qbf[:], src_ps[:])
            rt = rtp.tile([128, NCH], bf16, tag="rt")
            nc.vector.tensor_copy(rt[0:64, :], qbf[64:128, :])
            nc.vector.tensor_copy(rt[64:128, :], qbf[0:64, :])
            m1 = rtp.tile([128, NCH], bf16, tag="m1")
            nc.vector.tensor_mul(m1[:], qbf[:], cos_ap)
            nc.vector.tensor_mul(rt[:], rt[:], sin_ap)
            nc.vector.tensor_tensor(out=dst, in0=m1[:], in1=rt[:], op=OP.add)

        def emit_proj_qt(b, qt, alt):
            """One out-projection unit: 128 output rows x full C."""
            tok0 = b * T
            osb = outp.tile([128, C], bf16, tag="osb")
            for fc in range(C // NCH):
                ops = psum.tile([128, NCH], f32, tag="mm")
                for kd in range(NQH):
                    nc.tensor.matmul(
                        ops[:],
                        yT[:, kd, tok0 + qt * 128: tok0 + (qt + 1) * 128],
                        wp_sb[:, kd, ts(fc, NCH)],
                        start=(kd == 0), stop=(kd == NQH - 1))
                if fc % 2 == alt:
                    nc.scalar.copy(osb[:, ts(fc, NCH)], ops[:])
                else:
                    nc.vector.tensor_copy(osb[:, ts(fc, NCH)], ops[:])
            nc.sync.dma_start(
                out_e.ap()[tok0 + qt * 128: tok0 + (qt + 1) * 128, :],
                osb[:])

        pending = []
        for b in range(B):
            tok0 = b * T
            # ---- phase 1: QKV projection + RoPE for batch b ----
            if b == 0:
                xt_sb = xt0_sb  # DMAs already emitted up top
            else:
                xt_sb = xtp.tile([128, KT, T], bf16, tag="xt")
                for tc_ in range(QCH):
                    for kt in range(KT):
                        nc.sync.dma_start(
                            xt_sb[:, kt, ts(tc_, NCH)],
                            xt_e.ap()[kt * 128:(kt + 1) * 128,
                                      tok0 + tc_ * NCH: tok0 + (tc_ + 1) * NCH])
            for tc_ in range(QCH):
                cs = cos_sb[:, ts(tc_, NCH)]
                sn = sin_sb[:, ts(tc_, NCH)]
                for h in range(NQH):
                    ps = psum.tile([128, NCH], f32, tag="mm")
                    for kt in range(KT):
                        nc.tensor.matmul(ps[:],
                                         wq_sb[:, kt, h * D:(h + 1) * D],
                                         xt_sb[:, kt, ts(tc_, NCH)],
                                         start=(kt == 0), stop=(kt == KT - 1))
                    rope_out(qT[:, h, tok0 + tc_ * NCH: tok0 + (tc_ + 1) * NCH],
                             ps, cs, sn)
                ps = psum.tile([128, NCH], f32, tag="mm")
                for kt in range(KT):
                    nc.tensor.matmul(ps[:], wk_sb[:, kt, :],
                                     xt_sb[:, kt, ts(tc_, NCH)],
                                     start=(kt == 0), stop=(kt == KT - 1))
                rope_out(kT[:, tok0 + tc_ * NCH: tok0 + (tc_ + 1) * NCH],
                         ps, cs, sn)
                for ti in range(4 * tc_, 4 * tc_ + 4):
                    ps = psum.tile([128, D], f32, tag="mm")
                    for kt in range(KT):
                        nc.tensor.matmul(ps[:],
                                         xt_sb[:, kt, ti * 128:(ti + 1) * 128],
                                         wv_sb[:, kt, :],
                                         start=(kt == 0), stop=(kt == KT - 1))
                    nc.scalar.copy(vv[:, b * KB + ti, :], ps[:])
                # b1 phase 1 absorbs the (b0, qc3) out-projection: scalar
                # engine is otherwise idle here.
                if pending:
                    pending.pop(0)()

            # ---- phase 2+3: attention + out-projection for batch b ----
            # proj for chunk qc-1 is interleaved into chunk qc's attention
            # at k-tile granularity (keeps the ACT queue free for exp).
            for qc in range(QCH):
                n_kt = 4 * qc + 4
                stride = max(1, (2 * n_kt) // 5)
                tcount = 0
                for h in range(NQH):
                    yps = psum.tile([128, NCH], f32, tag="y", bufs=2)
                    sps = psum.tile([128, NCH], f32, tag="s", bufs=2)
                    for kti in range(n_kt):
                        dq = kti - 4 * qc
                        c0 = dq * 128 if dq > 0 else 0  # masked cols skipped
                        q_sl = qT[:, h, tok0 + qc * NCH + c0:
                                  tok0 + (qc + 1) * NCH]
                        sc = psum.tile([128, NCH], f32, tag="sc", bufs=2)
                        nc.tensor.matmul(sc[:, c0:],
                                         kT[:, tok0 + kti * 128: tok0 + (kti + 1) * 128],
                                         q_sl, start=True, stop=True)
                        ex = exp_p.tile([128, NCH], bf16, tag="ex")
                        nc.scalar.activation(ex[:, c0:], sc[:, c0:], AF.Exp)
                        if dq >= 0:
                            nc.vector.tensor_mul(ex[:, ts(dq, 128)],
                                                 ex[:, ts(dq, 128)], tri_sb[:])
                        st = (kti == 0)
                        sp = (kti == n_kt - 1)
                        nc.tensor.matmul(yps[:, c0:], vv[:, b * KB + kti, :],
                                         ex[:, c0:], start=st, stop=sp)
                        nc.tensor.matmul(sps[:, c0:], ones_sb[:], ex[:, c0:],
                                         start=st, stop=sp)
                        tcount += 1
                        if tcount % stride == 0 and pending:
                            pending.pop(0)()
                    rec = recp.tile([128, NCH], f32, tag="rec")
                    nc.vector.reciprocal(rec[:], sps[:])
                    nc.vector.tensor_mul(
                        yT[:, h, tok0 + qc * NCH: tok0 + (qc + 1) * NCH],
                        yps[:], rec[:])
                while pending:
                    pending.pop(0)()
                pending = [
                    (lambda b_=b, qt_=qt, alt_=qt % 2:
                     emit_proj_qt(b_, qt_, alt_))
                    for qt in range(4 * qc, 4 * qc + 4)]
        while pending:
            pending.pop(0)()

    nc.compile()
    return nc


def _get_nc():
    if "nc" not in _COMPILED:
        _COMPILED["nc"] = _build_nc()
    return _COMPILED["nc"]


def _stage_inputs(x, Wq, Wkv, Wproj):
    xt = np.ascontiguousarray(
        x.reshape(TOK, C).T).astype(BF16)                       # [C, TOK]
    cos, sin_s = _rope_tables()
    cos = cos.astype(BF16)
    sin_s = sin_s.astype(BF16)
    kk, qq = np.meshgrid(np.arange(D), np.arange(D), indexing="ij")
    tri = (kk <= qq).astype(BF16)                               # [k, q]

    in_maps = []
    for c in range(N_CORES):
        g = c // 2
        wq = np.ascontiguousarray(
            (Wq[2 * c * D:(2 * c + 2) * D, :] * SCALE).T).astype(BF16)
        wk = np.ascontiguousarray(Wkv[g * D:(g + 1) * D, :].T).astype(BF16)
        wv = np.ascontiguousarray(
            Wkv[4 * D + g * D: 4 * D + (g + 1) * D, :].T).astype(BF16)
        wp = np.ascontiguousarray(
            Wproj[:, 2 * c * D:(2 * c + 2) * D].T).astype(BF16)
        in_maps.append({
            "xt": xt, "wq": wq, "wk": wk, "wv": wv, "wp": wp,
            "cos": cos, "sin": sin_s, "tri": tri,
        })
    return in_maps


def run(x, Wq, Wkv, Wproj, trace=False):
    from concourse.bass_utils import run_bass_kernel_spmd

    nc = _get_nc()
    in_maps = _stage_inputs(x, Wq, Wkv, Wproj)
    res = run_bass_kernel_spmd(nc, in_maps, core_ids=list(range(N_CORES)),
                               trace=trace)
    acc = np.zeros((TOK, C), np.float32)
    for c in range(N_CORES):
        acc += np.asarray(res.results[c]["out"], np.float32)
    out = acc.reshape(B, T, C)
    return (out, res) if trace else (out, None)


def kernel(x, Wq, Wkv, Wproj):
    out, _ = run(np.asarray(x, np.float32), np.asarray(Wq, np.float32),
                 np.asarray(Wkv, np.float32), np.asarray(Wproj, np.float32))
    return out


# revision 8
# speedup vs baseline: 1.0928x; 1.0928x over previous
"""Causal GQA self-attention (B=2, T=2048, D=2048, 16 q-heads / 4 kv-heads,
head_dim=128, full-dim RoPE) on 8 Trainium2 NeuronCores.

Strategy: tensor-parallel over heads. Core c owns q-heads {2c, 2c+1} and
kv-head c//2. Wq/Wkv output dims and Wproj input dims are sharded 8-ways on
the host; each core computes a full-width partial of the output projection
(bf16) and the host sums the 8 partials in f32.

On-chip layout: x is staged transposed (channel-major [C, B*T]) so the
QKV projections need no on-chip transpose; attention is computed "k-major"
(scores transposed, [k_pos, q_pos]) so the P@V contraction needs no
transpose either. Softmax runs without max-subtraction (scores are ~N(0,1);
exp never overflows) and the denominator comes from an all-ones stationary
matmul which also broadcasts the per-column sums across all partitions.

Scheduling notes (v2):
- RoPE runs in bf16: the projection PSUM is staged to SBUF via an ACT copy
  (scalar engine is idle in phase 1) so every DVE op is all-bf16/SBUF and
  hits the 2x perf mode. The rotate-half sign is folded into the sin table.
- Output partials are written bf16 (halves the 33MB write DMA).
- The out-projection for chunk qc-1 is interleaved into chunk qc's
  attention stream at k-tile granularity so its PSUM->SBUF copies never
  form a burst ahead of the next exp in the ACT queue.
- PSUM tags: mm(2) sc(2) y(2) s(2) = 8 banks, no cross-phase sharing.
- DMA emission order: wq/xt-chunk0 interleaved per k-tile, then cos/sin,
  wk/wv, remaining chunks, tri, wp last; batch-1 xt is emitted right after
  batch-0's QKV loop so it streams during batch-0 attention.
"""

import math
import os
import sys

for _p in ("/opt/trn_rl_repo", "/root/.axon_site/_ro/trn_rl_repo"):
    if os.path.isdir(_p) and _p not in sys.path:
        sys.path.insert(0, _p)

import ml_dtypes
import numpy as np

BF16 = ml_dtypes.bfloat16

B = 2
T = 2048
C = 2048
D = 128          # head dim
NQH = 2          # q heads per core
TOK = B * T      # 4096
KT = C // 128    # 16 contraction tiles
NCH = 512        # matmul moving-dim chunk
QCH = T // NCH   # 4 q chunks per batch
KB = T // 128    # 16 k tiles per batch
N_CORES = 8
SCALE = 1.0 / math.sqrt(D)

_COMPILED = {}


def _rope_tables():
    dim = np.arange(D // 2, dtype=np.float64)
    freq = 10000.0 ** (dim / (D / 2))
    freq = np.concatenate([freq, freq])              # [128]
    pos = np.arange(T, dtype=np.float64)
    ang = pos[None, :] / freq[:, None]               # [128, T] channel-major
    cos = np.cos(ang)
    sin_s = np.sin(ang)
    sin_s[: D // 2] = -sin_s[: D // 2]               # rotate-half sign folded
    return cos, sin_s


def _build_nc(debug=False):
    import concourse.bass as bass  # noqa: F401
    import concourse.mybir as mybir
    import concourse.tile as tile
    from concourse import bacc
    from concourse.bass import ts

    f32 = mybir.dt.float32
    bf16 = mybir.dt.bfloat16
    AF = mybir.ActivationFunctionType
    OP = mybir.AluOpType

    nc = bacc.Bacc("TRN2", target_bir_lowering=False, debug=False,
                   num_devices=N_CORES)

    xt_e = nc.dram_tensor("xt", [C, TOK], bf16, kind="ExternalInput")
    wq_e = nc.dram_tensor("wq", [C, NQH * D], bf16, kind="ExternalInput")
    wk_e = nc.dram_tensor("wk", [C, D], bf16, kind="ExternalInput")
    wv_e = nc.dram_tensor("wv", [C, D], bf16, kind="ExternalInput")
    wp_e = nc.dram_tensor("wp", [NQH * D, C], bf16, kind="ExternalInput")
    cos_e = nc.dram_tensor("cos", [D, T], bf16, kind="ExternalInput")
    sin_e = nc.dram_tensor("sin", [D, T], bf16, kind="ExternalInput")
    tri_e = nc.dram_tensor("tri", [D, D], bf16, kind="ExternalInput")
    out_e = nc.dram_tensor("out", [TOK, C], bf16, kind="ExternalOutput")

    from contextlib import ExitStack

    with tile.TileContext(nc) as tc, ExitStack() as ctx:
        const = ctx.enter_context(tc.tile_pool(name="const", bufs=1))
        qkvp = ctx.enter_context(tc.tile_pool(name="qkv", bufs=1))
        psum = ctx.enter_context(tc.tile_pool(name="ps", bufs=2, space="PSUM"))
        xtp = ctx.enter_context(tc.tile_pool(name="xt", bufs=5))
        w1p = ctx.enter_context(tc.tile_pool(name="w1", bufs=1))
        rtp = ctx.enter_context(tc.tile_pool(name="rt", bufs=2))
        stag = ctx.enter_context(tc.tile_pool(name="stg", bufs=3))
        exp_p = ctx.enter_context(tc.tile_pool(name="exp", bufs=6))
        recp = ctx.enter_context(tc.tile_pool(name="rec", bufs=2))
        outp = ctx.enter_context(tc.tile_pool(name="outs", bufs=3))

        # ---- DMA emission order: get the first q-proj matmul running as
        # early as possible (wq/xt interleaved per k-tile), then the tables
        # needed by the first rope, then the rest in consumption order.
        xt_tiles = {}

        def load_xt_chunk(b, c):
            """One [C, NCH] token-chunk of x, channel-major, per-kt DMAs."""
            t = xtp.tile([128, KT, NCH], bf16, tag="xt")
            xt_tiles[(b, c)] = t
            for kt in range(KT):
                nc.sync.dma_start(
                    t[:, kt, :],
                    xt_e.ap()[kt * 128:(kt + 1) * 128,
                              b * T + c * NCH: b * T + (c + 1) * NCH])

        wq_sb = w1p.tile([128, KT, NQH * D], bf16, tag="wq")
        xt00 = xtp.tile([128, KT, NCH], bf16, tag="xt")
        xt_tiles[(0, 0)] = xt00
        for kt in range(KT):
            nc.sync.dma_start(wq_sb[:, kt, :],
                              wq_e.ap()[kt * 128:(kt + 1) * 128, :])
            nc.sync.dma_start(xt00[:, kt, :],
                              xt_e.ap()[kt * 128:(kt + 1) * 128, 0:NCH])
        cos_sb = const.tile([D, T], bf16, tag="cos")
        nc.sync.dma_start(cos_sb[:], cos_e.ap())
        sin_sb = const.tile([D, T], bf16, tag="sin")
        nc.sync.dma_start(sin_sb[:], sin_e.ap())
        wk_sb = w1p.tile([128, KT, D], bf16, tag="wk")
        nc.sync.dma_start(wk_sb[:], wk_e.ap().rearrange("(ko p) n -> p ko n", p=128))
        wv_sb = w1p.tile([128, KT, D], bf16, tag="wv")
        nc.sync.dma_start(wv_sb[:], wv_e.ap().rearrange("(ko p) n -> p ko n", p=128))
        for tc_ in range(1, QCH):
            load_xt_chunk(0, tc_)
        tri_sb = const.tile([D, D], bf16, tag="tri")
        nc.sync.dma_start(tri_sb[:], tri_e.ap())
        ones_sb = const.tile([128, 128], bf16, tag="ones")
        nc.vector.memset(ones_sb[:], 1.0)
        wp_sb = const.tile([128, NQH, C], bf16, tag="wp")
        nc.sync.dma_start(wp_sb[:], wp_e.ap().rearrange("(ko p) n -> p ko n", p=128))

        # persistent per-batch-pair tensors
        qT = qkvp.tile([D, NQH, TOK], bf16, tag="qT")    # rope'd, pre-scaled
        kT = qkvp.tile([D, TOK], bf16, tag="kT")         # rope'd
        vv = qkvp.tile([128, B * KB, D], bf16, tag="vv")  # token-major
        yT = qkvp.tile([D, NQH, TOK], bf16, tag="yT")    # attn out, normalized

        def rope_out(dst, src_ps, cos_ap, sin_ap):
            """dst(bf16) = src * cos + rotate_half(src) * sin_signed.

            src is staged PSUM->SBUF bf16 on the scalar engine so all DVE
            ops below run in the 2x/4x (all-bf16, all-SBUF) perf modes.
            tensor_tensor operands must share a start partition, so the
            rotate-half is done with two (partition-shift-capable) copies."""
            qbf = stag.tile([128, NCH], bf16, tag="qbf")
            nc.scalar.copy(qbf[:], src_ps[:])
            rt = rtp.tile([128, NCH], bf16, tag="rt")
            nc.vector.tensor_copy(rt[0:64, :], qbf[64:128, :])
            nc.vector.tensor_copy(rt[64:128, :], qbf[0:64, :])
            m1 = rtp.tile([128, NCH], bf16, tag="m1")
            nc.vector.tensor_mul(m1[:], qbf[:], cos_ap)
            nc.vector.tensor_mul(rt[:], rt[:], sin_ap)
            nc.vector.tensor_tensor(out=dst, in0=m1[:], in1=rt[:], op=OP.add)

        def emit_proj_qt(b, qt, alt):
            """One out-projection unit: 128 output rows x full C."""
            tok0 = b * T
            osb = outp.tile([128, C], bf16, tag="osb")
            for fc in range(C // NCH):
                ops = psum.tile([128, NCH], f32, tag="mm")
                for kd in range(NQH):
                    nc.tensor.matmul(
                        ops[:],
                        yT[:, kd, tok0 + qt * 128: tok0 + (qt + 1) * 128],
                        wp_sb[:, kd, ts(fc, NCH)],
                        start=(kd == 0), stop=(kd == NQH - 1))
                if fc % 2 == alt:
                    nc.scalar.copy(osb[:, ts(fc, NCH)], ops[:])
                else:
                    nc.vector.tensor_copy(osb[:, ts(fc, NCH)], ops[:])
            nc.sync.dma_start(
                out_e.ap()[tok0 + qt * 128: tok0 + (qt + 1) * 128, :],
                osb[:])

        pending = []
        for b in range(B):
            tok0 = b * T
            # Per chunk: QKV projection + RoPE, then immediately the causal
            # attention for that q-chunk (its k/v prefix just completed) with
            # the previous chunk's out-projection interleaved at k-tile
            # granularity. This keeps the PE fed while xt DMA streams in and
            # never leaves a >3.4us idle window (HAM stays warm).
            for qc in range(QCH):
                xt_sb = xt_tiles[(b, qc)]
                cs = cos_sb[:, ts(qc, NCH)]
                sn = sin_sb[:, ts(qc, NCH)]
                for h in range(NQH):
                    ps = psum.tile([128, NCH], f32, tag="mm")
                    for kt in range(KT):
                        nc.tensor.matmul(ps[:],
                                         wq_sb[:, kt, h * D:(h + 1) * D],
                                         xt_sb[:, kt, :],
                                         start=(kt == 0), stop=(kt == KT - 1))
                    rope_out(qT[:, h, tok0 + qc * NCH: tok0 + (qc + 1) * NCH],
                             ps, cs, sn)
                ps = psum.tile([128, NCH], f32, tag="mm")
                for kt in range(KT):
                    nc.tensor.matmul(ps[:], wk_sb[:, kt, :],
                                     xt_sb[:, kt, :],
                                     start=(kt == 0), stop=(kt == KT - 1))
                rope_out(kT[:, tok0 + qc * NCH: tok0 + (qc + 1) * NCH],
                         ps, cs, sn)
                for ti in range(4):
                    ps = psum.tile([128, D], f32, tag="mm")
                    for kt in range(KT):
                        nc.tensor.matmul(ps[:],
                                         xt_sb[:, kt, ti * 128:(ti + 1) * 128],
                                         wv_sb[:, kt, :],
                                         start=(kt == 0), stop=(kt == KT - 1))
                    nc.scalar.copy(vv[:, b * KB + 4 * qc + ti, :], ps[:])
                if b == 0:
                    # prefetch batch 1's chunk: its WAR on this buffer just
                    # cleared, so it streams ~50us ahead of its use.
                    load_xt_chunk(1, qc)

                n_kt = 4 * qc + 4
                stride = max(1, (2 * n_kt) // 5)
                tcount = 0
                for h in range(NQH):
                    yps = psum.tile([128, NCH], f32, tag="y", bufs=2)
                    sps = psum.tile([128, NCH], f32, tag="s", bufs=2)
                    for kti in range(n_kt):
                        dq = kti - 4 * qc
                        c0 = dq * 128 if dq > 0 else 0  # masked cols skipped
                        q_sl = qT[:, h, tok0 + qc * NCH + c0:
                                  tok0 + (qc + 1) * NCH]
                        sc = psum.tile([128, NCH], f32, tag="sc", bufs=2)
                        nc.tensor.matmul(sc[:, c0:],
                                         kT[:, tok0 + kti * 128: tok0 + (kti + 1) * 128],
                                         q_sl, start=True, stop=True)
                        ex = exp_p.tile([128, NCH], bf16, tag="ex")
                        nc.scalar.activation(ex[:, c0:], sc[:, c0:], AF.Exp)
                        if dq >= 0:
                            nc.vector.tensor_mul(ex[:, ts(dq, 128)],
                                                 ex[:, ts(dq, 128)], tri_sb[:])
                        st = (kti == 0)
                        sp = (kti == n_kt - 1)
                        nc.tensor.matmul(yps[:, c0:], vv[:, b * KB + kti, :],
                                         ex[:, c0:], start=st, stop=sp)
                        nc.tensor.matmul(sps[:, c0:], ones_sb[:], ex[:, c0:],
                                         start=st, stop=sp)
                        tcount += 1
                        if tcount % stride == 0 and pending:
                            pending.pop(0)()
                    rec = recp.tile([128, NCH], f32, tag="rec")
                    nc.vector.reciprocal_approx_fast(out=rec[:], in_=sps[:])
                    nc.vector.tensor_mul(
                        yT[:, h, tok0 + qc * NCH: tok0 + (qc + 1) * NCH],
                        yps[:], rec[:])
                while pending:
                    pending.pop(0)()
                pending = [
                    (lambda b_=b, qt_=qt, alt_=qt % 2:
                     emit_proj_qt(b_, qt_, alt_))
                    for qt in range(4 * qc, 4 * qc + 4)]
        while pending:
            pending.pop(0)()

    nc.compile()
    return nc


def _get_nc():
    if "nc" not in _COMPILED:
        _COMPILED["nc"] = _build_nc()
    return _COMPILED["nc"]


def _stage_inputs(x, Wq, Wkv, Wproj):
    xt = np.ascontiguousarray(
        x.reshape(TOK, C).T).astype(BF16)                       # [C, TOK]
    cos, sin_s = _rope_tables()
    cos = cos.astype(BF16)
    sin_s = sin_s.astype(BF16)
    kk, qq = np.meshgrid(np.arange(D), np.arange(D), indexing="ij")
    tri = (kk <= qq).astype(BF16)                               # [k, q]

    in_maps = []
    for c in range(N_CORES):
        g = c // 2
        wq = np.ascontiguousarray(
            (Wq[2 * c * D:(2 * c + 2) * D, :] * SCALE).T).astype(BF16)
        wk = np.ascontiguousarray(Wkv[g * D:(g + 1) * D, :].T).astype(BF16)
        wv = np.ascontiguousarray(
            Wkv[4 * D + g * D: 4 * D + (g + 1) * D, :].T).astype(BF16)
        wp = np.ascontiguousarray(
            Wproj[:, 2 * c * D:(2 * c + 2) * D].T).astype(BF16)
        in_maps.append({
            "xt": xt, "wq": wq, "wk": wk, "wv": wv, "wp": wp,
            "cos": cos, "sin": sin_s, "tri": tri,
        })
    return in_maps


def run(x, Wq, Wkv, Wproj, trace=False):
    from concourse.bass_utils import run_bass_kernel_spmd

    nc = _get_nc()
    in_maps = _stage_inputs(x, Wq, Wkv, Wproj)
    res = run_bass_kernel_spmd(nc, in_maps, core_ids=list(range(N_CORES)),
                               trace=trace)
    acc = np.zeros((TOK, C), np.float32)
    for c in range(N_CORES):
        acc += np.asarray(res.results[c]["out"], np.float32)
    out = acc.reshape(B, T, C)
    return (out, res) if trace else (out, None)


def kernel(x, Wq, Wkv, Wproj):
    out, _ = run(np.asarray(x, np.float32), np.asarray(Wq, np.float32),
                 np.asarray(Wkv, np.float32), np.asarray(Wproj, np.float32))
    return out


# revision 9
# speedup vs baseline: 1.1907x; 1.0895x over previous
"""Causal GQA self-attention (B=2, T=2048, D=2048, 16 q-heads / 4 kv-heads,
head_dim=128, full-dim RoPE) on 8 Trainium2 NeuronCores.

Strategy: tensor-parallel over heads. Core c owns q-heads {2c, 2c+1} and
kv-head c//2. Wq/Wkv output dims and Wproj input dims are sharded 8-ways on
the host; each core computes a full-width partial of the output projection
(bf16) and the host sums the 8 partials in f32.

On-chip layout: x is staged transposed (channel-major [C, B*T]) so the
QKV projections need no on-chip transpose; attention is computed "k-major"
(scores transposed, [k_pos, q_pos]) so the P@V contraction needs no
transpose either. Softmax runs without max-subtraction (scores are ~N(0,1);
exp never overflows) and the denominator comes from an all-ones stationary
matmul which also broadcasts the per-column sums across all partitions.

Scheduling notes (v2):
- RoPE runs in bf16: the projection PSUM is staged to SBUF via an ACT copy
  (scalar engine is idle in phase 1) so every DVE op is all-bf16/SBUF and
  hits the 2x perf mode. The rotate-half sign is folded into the sin table.
- Output partials are written bf16 (halves the 33MB write DMA).
- The out-projection for chunk qc-1 is interleaved into chunk qc's
  attention stream at k-tile granularity so its PSUM->SBUF copies never
  form a burst ahead of the next exp in the ACT queue.
- PSUM tags: mm(2) sc(2) y(2) s(2) = 8 banks, no cross-phase sharing.
- DMA emission order: wq/xt-chunk0 interleaved per k-tile, then cos/sin,
  wk/wv, remaining chunks, tri, wp last; batch-1 xt is emitted right after
  batch-0's QKV loop so it streams during batch-0 attention.
"""

import math
import os
import sys

for _p in ("/opt/trn_rl_repo", "/root/.axon_site/_ro/trn_rl_repo"):
    if os.path.isdir(_p) and _p not in sys.path:
        sys.path.insert(0, _p)

import ml_dtypes
import numpy as np

BF16 = ml_dtypes.bfloat16

B = 2
T = 2048
C = 2048
D = 128          # head dim
NQH = 2          # q heads per core
TOK = B * T      # 4096
KT = C // 128    # 16 contraction tiles
NCH = 512        # matmul moving-dim chunk
QCH = T // NCH   # 4 q chunks per batch
KB = T // 128    # 16 k tiles per batch
N_CORES = 8
SCALE = 1.0 / math.sqrt(D)

_COMPILED = {}


def _rope_tables():
    dim = np.arange(D // 2, dtype=np.float64)
    freq = 10000.0 ** (dim / (D / 2))
    freq = np.concatenate([freq, freq])              # [128]
    pos = np.arange(T, dtype=np.float64)
    ang = pos[None, :] / freq[:, None]               # [128, T] channel-major
    cos = np.cos(ang)
    sin_s = np.sin(ang)
    sin_s[: D // 2] = -sin_s[: D // 2]               # rotate-half sign folded
    return cos, sin_s


def _build_nc(debug=False):
    import concourse.bass as bass  # noqa: F401
    import concourse.mybir as mybir
    import concourse.tile as tile
    from concourse import bacc
    from concourse.bass import ts

    f32 = mybir.dt.float32
    bf16 = mybir.dt.bfloat16
    AF = mybir.ActivationFunctionType
    OP = mybir.AluOpType

    nc = bacc.Bacc("TRN2", target_bir_lowering=False, debug=False,
                   num_devices=N_CORES)

    xt_e = nc.dram_tensor("xt", [C, TOK], bf16, kind="ExternalInput")
    wq_e = nc.dram_tensor("wq", [C, NQH * D], bf16, kind="ExternalInput")
    wk_e = nc.dram_tensor("wk", [C, D], bf16, kind="ExternalInput")
    wv_e = nc.dram_tensor("wv", [C, D], bf16, kind="ExternalInput")
    wp_e = nc.dram_tensor("wp", [NQH * D, C], bf16, kind="ExternalInput")
    cos_e = nc.dram_tensor("cos", [D, T], bf16, kind="ExternalInput")
    sin_e = nc.dram_tensor("sin", [D, T], bf16, kind="ExternalInput")
    tri_e = nc.dram_tensor("tri", [D, D], bf16, kind="ExternalInput")
    out_e = nc.dram_tensor("out", [TOK, C], bf16, kind="ExternalOutput")

    from contextlib import ExitStack

    with tile.TileContext(nc) as tc, ExitStack() as ctx:
        const = ctx.enter_context(tc.tile_pool(name="const", bufs=1))
        qkvp = ctx.enter_context(tc.tile_pool(name="qkv", bufs=1))
        psum = ctx.enter_context(tc.tile_pool(name="ps", bufs=2, space="PSUM"))
        xtp = ctx.enter_context(tc.tile_pool(name="xt", bufs=5))
        w1p = ctx.enter_context(tc.tile_pool(name="w1", bufs=1))
        rtp = ctx.enter_context(tc.tile_pool(name="rt", bufs=2))
        stag = ctx.enter_context(tc.tile_pool(name="stg", bufs=3))
        exp_p = ctx.enter_context(tc.tile_pool(name="exp", bufs=6))
        recp = ctx.enter_context(tc.tile_pool(name="rec", bufs=2))
        outp = ctx.enter_context(tc.tile_pool(name="outs", bufs=3))

        # ---- DMA emission order: get the first q-proj matmul running as
        # early as possible (wq/xt interleaved per k-tile), then the tables
        # needed by the first rope, then the rest in consumption order.
        xt_tiles = {}

        def load_xt_chunk(b, c):
            """One [C, NCH] token-chunk of x, channel-major, one DMA issue
            (the sync engine serializes dma_start issue at ~650ns each, so
            instruction count matters more than transfer granularity)."""
            t = xtp.tile([128, KT, NCH], bf16, tag="xt")
            xt_tiles[(b, c)] = t
            nc.sync.dma_start(
                t[:],
                xt_e.ap()[:, b * T + c * NCH: b * T + (c + 1) * NCH]
                .rearrange("(ko p) n -> p ko n", p=128))

        wq_sb = w1p.tile([128, KT, NQH * D], bf16, tag="wq")
        nc.sync.dma_start(wq_sb[:], wq_e.ap().rearrange("(ko p) n -> p ko n", p=128))
        load_xt_chunk(0, 0)
        cos_sb = const.tile([D, T], bf16, tag="cos")
        nc.sync.dma_start(cos_sb[:], cos_e.ap())
        sin_sb = const.tile([D, T], bf16, tag="sin")
        nc.sync.dma_start(sin_sb[:], sin_e.ap())
        tri_sb = const.tile([D, D], bf16, tag="tri")
        nc.sync.dma_start(tri_sb[:], tri_e.ap())
        ones_sb = const.tile([128, 128], bf16, tag="ones")
        nc.vector.memset(ones_sb[:], 1.0)
        wk_sb = w1p.tile([128, KT, D], bf16, tag="wk")
        nc.sync.dma_start(wk_sb[:], wk_e.ap().rearrange("(ko p) n -> p ko n", p=128))
        wv_sb = w1p.tile([128, KT, D], bf16, tag="wv")
        nc.sync.dma_start(wv_sb[:], wv_e.ap().rearrange("(ko p) n -> p ko n", p=128))
        wp_sb = const.tile([128, NQH, C], bf16, tag="wp")
        nc.sync.dma_start(wp_sb[:], wp_e.ap().rearrange("(ko p) n -> p ko n", p=128))
        for tc_ in range(1, QCH):
            load_xt_chunk(0, tc_)

        # persistent per-batch-pair tensors
        qT = qkvp.tile([D, NQH, TOK], bf16, tag="qT")    # rope'd, pre-scaled
        kT = qkvp.tile([D, TOK], bf16, tag="kT")         # rope'd
        vv = qkvp.tile([128, B * KB, D], bf16, tag="vv")  # token-major
        yT = qkvp.tile([D, NQH, TOK], bf16, tag="yT")    # attn out, normalized

        def rope_out(dst, src_ps, cos_ap, sin_ap):
            """dst(bf16) = src * cos + rotate_half(src) * sin_signed.

            src is staged PSUM->SBUF bf16 on the scalar engine so all DVE
            ops below run in the 2x/4x (all-bf16, all-SBUF) perf modes.
            tensor_tensor operands must share a start partition, so the
            rotate-half is done with two (partition-shift-capable) copies."""
            qbf = stag.tile([128, NCH], bf16, tag="qbf")
            nc.scalar.copy(qbf[:], src_ps[:])
            rt = rtp.tile([128, NCH], bf16, tag="rt")
            nc.vector.tensor_copy(rt[0:64, :], qbf[64:128, :])
            nc.vector.tensor_copy(rt[64:128, :], qbf[0:64, :])
            m1 = rtp.tile([128, NCH], bf16, tag="m1")
            nc.vector.tensor_mul(m1[:], qbf[:], cos_ap)
            nc.vector.tensor_mul(rt[:], rt[:], sin_ap)
            nc.vector.tensor_tensor(out=dst, in0=m1[:], in1=rt[:], op=OP.add)

        def emit_proj_qt(b, qt, alt):
            """One out-projection unit: 128 output rows x full C."""
            tok0 = b * T
            osb = outp.tile([128, C], bf16, tag="osb")
            for fc in range(C // NCH):
                ops = psum.tile([128, NCH], f32, tag="mm")
                for kd in range(NQH):
                    nc.tensor.matmul(
                        ops[:],
                        yT[:, kd, tok0 + qt * 128: tok0 + (qt + 1) * 128],
                        wp_sb[:, kd, ts(fc, NCH)],
                        start=(kd == 0), stop=(kd == NQH - 1))
                if fc % 2 == alt:
                    nc.scalar.copy(osb[:, ts(fc, NCH)], ops[:])
                else:
                    nc.vector.tensor_copy(osb[:, ts(fc, NCH)], ops[:])
            nc.sync.dma_start(
                out_e.ap()[tok0 + qt * 128: tok0 + (qt + 1) * 128, :],
                osb[:])

        pending = []
        for b in range(B):
            tok0 = b * T
            # Per chunk: QKV projection + RoPE, then immediately the causal
            # attention for that q-chunk (its k/v prefix just completed) with
            # the previous chunk's out-projection interleaved at k-tile
            # granularity. This keeps the PE fed while xt DMA streams in and
            # never leaves a >3.4us idle window (HAM stays warm).
            for qc in range(QCH):
                xt_sb = xt_tiles[(b, qc)]
                cs = cos_sb[:, ts(qc, NCH)]
                sn = sin_sb[:, ts(qc, NCH)]
                for h in range(NQH):
                    ps = psum.tile([128, NCH], f32, tag="mm")
                    for kt in range(KT):
                        nc.tensor.matmul(ps[:],
                                         wq_sb[:, kt, h * D:(h + 1) * D],
                                         xt_sb[:, kt, :],
                                         start=(kt == 0), stop=(kt == KT - 1))
                    rope_out(qT[:, h, tok0 + qc * NCH: tok0 + (qc + 1) * NCH],
                             ps, cs, sn)
                ps = psum.tile([128, NCH], f32, tag="mm")
                for kt in range(KT):
                    nc.tensor.matmul(ps[:], wk_sb[:, kt, :],
                                     xt_sb[:, kt, :],
                                     start=(kt == 0), stop=(kt == KT - 1))
                rope_out(kT[:, tok0 + qc * NCH: tok0 + (qc + 1) * NCH],
                         ps, cs, sn)
                for ti in range(4):
                    ps = psum.tile([128, D], f32, tag="mm")
                    for kt in range(KT):
                        nc.tensor.matmul(ps[:],
                                         xt_sb[:, kt, ti * 128:(ti + 1) * 128],
                                         wv_sb[:, kt, :],
                                         start=(kt == 0), stop=(kt == KT - 1))
                    nc.scalar.copy(vv[:, b * KB + 4 * qc + ti, :], ps[:])
                if b == 0:
                    # prefetch batch 1's chunk: its WAR on this buffer just
                    # cleared, so it streams ~50us ahead of its use.
                    load_xt_chunk(1, qc)

                n_kt = 4 * qc + 4
                stride = max(1, (2 * n_kt) // 5)
                tcount = 0
                for h in range(NQH):
                    yps = psum.tile([128, NCH], f32, tag="y", bufs=2)
                    sps = psum.tile([128, NCH], f32, tag="s", bufs=2)
                    for kti in range(n_kt):
                        dq = kti - 4 * qc
                        c0 = dq * 128 if dq > 0 else 0  # masked cols skipped
                        q_sl = qT[:, h, tok0 + qc * NCH + c0:
                                  tok0 + (qc + 1) * NCH]
                        sc = psum.tile([128, NCH], f32, tag="sc", bufs=2)
                        nc.tensor.matmul(sc[:, c0:],
                                         kT[:, tok0 + kti * 128: tok0 + (kti + 1) * 128],
                                         q_sl, start=True, stop=True)
                        ex = exp_p.tile([128, NCH], bf16, tag="ex")
                        nc.scalar.activation(ex[:, c0:], sc[:, c0:], AF.Exp)
                        if dq >= 0:
                            nc.vector.tensor_mul(ex[:, ts(dq, 128)],
                                                 ex[:, ts(dq, 128)], tri_sb[:])
                        st = (kti == 0)
                        sp = (kti == n_kt - 1)
                        nc.tensor.matmul(yps[:, c0:], vv[:, b * KB + kti, :],
                                         ex[:, c0:], start=st, stop=sp)
                        nc.tensor.matmul(sps[:, c0:], ones_sb[:], ex[:, c0:],
                                         start=st, stop=sp)
                        tcount += 1
                        if tcount % stride == 0 and pending:
                            pending.pop(0)()
                    rec = recp.tile([128, NCH], f32, tag="rec")
                    nc.vector.reciprocal_approx_fast(out=rec[:], in_=sps[:])
                    nc.vector.tensor_mul(
                        yT[:, h, tok0 + qc * NCH: tok0 + (qc + 1) * NCH],
                        yps[:], rec[:])
                while pending:
                    pending.pop(0)()
                pending = [
                    (lambda b_=b, qt_=qt, alt_=qt % 2:
                     emit_proj_qt(b_, qt_, alt_))
                    for qt in range(4 * qc, 4 * qc + 4)]
        while pending:
            pending.pop(0)()

    nc.compile()
    return nc


def _get_nc():
    if "nc" not in _COMPILED:
        _COMPILED["nc"] = _build_nc()
    return _COMPILED["nc"]


def _stage_inputs(x, Wq, Wkv, Wproj):
    xt = np.ascontiguousarray(
        x.reshape(TOK, C).T).astype(BF16)                       # [C, TOK]
    cos, sin_s = _rope_tables()
    cos = cos.astype(BF16)
    sin_s = sin_s.astype(BF16)
    kk, qq = np.meshgrid(np.arange(D), np.arange(D), indexing="ij")
    tri = (kk <= qq).astype(BF16)                               # [k, q]

    in_maps = []
    for c in range(N_CORES):
        g = c // 2
        wq = np.ascontiguousarray(
            (Wq[2 * c * D:(2 * c + 2) * D, :] * SCALE).T).astype(BF16)
        wk = np.ascontiguousarray(Wkv[g * D:(g + 1) * D, :].T).astype(BF16)
        wv = np.ascontiguousarray(
            Wkv[4 * D + g * D: 4 * D + (g + 1) * D, :].T).astype(BF16)
        wp = np.ascontiguousarray(
            Wproj[:, 2 * c * D:(2 * c + 2) * D].T).astype(BF16)
        in_maps.append({
            "xt": xt, "wq": wq, "wk": wk, "wv": wv, "wp": wp,
            "cos": cos, "sin": sin_s, "tri": tri,
        })
    return in_maps


def run(x, Wq, Wkv, Wproj, trace=False):
    from concourse.bass_utils import run_bass_kernel_spmd

    nc = _get_nc()
    in_maps = _stage_inputs(x, Wq, Wkv, Wproj)
    res = run_bass_kernel_spmd(nc, in_maps, core_ids=list(range(N_CORES)),
                               trace=trace)
    acc = np.zeros((TOK, C), np.float32)
    for c in range(N_CORES):
        acc += np.asarray(res.results[c]["out"], np.float32)
    out = acc.reshape(B, T, C)
    return (out, res) if trace else (out, None)


def kernel(x, Wq, Wkv, Wproj):
    out, _ = run(np.asarray(x, np.float32), np.asarray(Wq, np.float32),
                 np.asarray(Wkv, np.float32), np.asarray(Wproj, np.float32))
    return out


# revision 12
# speedup vs baseline: 1.2093x; 1.0156x over previous
"""Causal GQA self-attention (B=2, T=2048, D=2048, 16 q-heads / 4 kv-heads,
head_dim=128, full-dim RoPE) on 8 Trainium2 NeuronCores.

Strategy: tensor-parallel over heads. Core c owns q-heads {2c, 2c+1} and
kv-head c//2. Wq/Wkv output dims and Wproj input dims are sharded 8-ways on
the host; each core computes a full-width partial of the output projection
(bf16) and the host sums the 8 partials in f32.

On-chip layout: x is staged transposed (channel-major [C, B*T]) so the
QKV projections need no on-chip transpose; attention is computed "k-major"
(scores transposed, [k_pos, q_pos]) so the P@V contraction needs no
transpose either. Softmax runs without max-subtraction (scores are ~N(0,1);
exp never overflows) and the denominator comes from an all-ones stationary
matmul which also broadcasts the per-column sums across all partitions.

Scheduling notes (v2):
- RoPE runs in bf16: the projection PSUM is staged to SBUF via an ACT copy
  (scalar engine is idle in phase 1) so every DVE op is all-bf16/SBUF and
  hits the 2x perf mode. The rotate-half sign is folded into the sin table.
- Output partials are written bf16 (halves the 33MB write DMA).
- The out-projection for chunk qc-1 is interleaved into chunk qc's
  attention stream at k-tile granularity so its PSUM->SBUF copies never
  form a burst ahead of the next exp in the ACT queue.
- PSUM tags: mm(2) sc(2) y(2) s(2) = 8 banks, no cross-phase sharing.
- DMA emission order: wq/xt-chunk0 interleaved per k-tile, then cos/sin,
  wk/wv, remaining chunks, tri, wp last; batch-1 xt is emitted right after
  batch-0's QKV loop so it streams during batch-0 attention.
"""

import math
import os
import sys

for _p in ("/opt/trn_rl_repo", "/root/.axon_site/_ro/trn_rl_repo"):
    if os.path.isdir(_p) and _p not in sys.path:
        sys.path.insert(0, _p)

import ml_dtypes
import numpy as np

BF16 = ml_dtypes.bfloat16

B = 2
T = 2048
C = 2048
D = 128          # head dim
NQH = 2          # q heads per core
TOK = B * T      # 4096
KT = C // 128    # 16 contraction tiles
NCH = 512        # matmul moving-dim chunk
QCH = T // NCH   # 4 q chunks per batch
KB = T // 128    # 16 k tiles per batch
N_CORES = 8
SCALE = 1.0 / math.sqrt(D)

_COMPILED = {}


def _rope_tables():
    dim = np.arange(D // 2, dtype=np.float64)
    freq = 10000.0 ** (dim / (D / 2))
    freq = np.concatenate([freq, freq])              # [128]
    pos = np.arange(T, dtype=np.float64)
    ang = pos[None, :] / freq[:, None]               # [128, T] channel-major
    cos = np.cos(ang)
    sin_s = np.sin(ang)
    sin_s[: D // 2] = -sin_s[: D // 2]               # rotate-half sign folded
    return cos, sin_s


def _build_nc(debug=False):
    import concourse.bass as bass  # noqa: F401
    import concourse.mybir as mybir
    import concourse.tile as tile
    from concourse import bacc
    from concourse.bass import ts

    f32 = mybir.dt.float32
    bf16 = mybir.dt.bfloat16
    AF = mybir.ActivationFunctionType
    OP = mybir.AluOpType

    nc = bacc.Bacc("TRN2", target_bir_lowering=False, debug=False,
                   num_devices=N_CORES)

    # all inputs are host-pre-tiled to the exact SBUF layout so every DMA
    # moves long contiguous rows (16KB/partition) at full HBM bandwidth
    xt_e = nc.dram_tensor("xt", [128, B * QCH, KT, NCH], bf16,
                          kind="ExternalInput")
    wq_e = nc.dram_tensor("wq", [128, KT, NQH * D], bf16, kind="ExternalInput")
    wk_e = nc.dram_tensor("wk", [128, KT, D], bf16, kind="ExternalInput")
    wv_e = nc.dram_tensor("wv", [128, KT, D], bf16, kind="ExternalInput")
    wp_e = nc.dram_tensor("wp", [128, NQH, C], bf16, kind="ExternalInput")
    cos_e = nc.dram_tensor("cos", [D, T], bf16, kind="ExternalInput")
    sin_e = nc.dram_tensor("sin", [D, T], bf16, kind="ExternalInput")
    tri_e = nc.dram_tensor("tri", [D, D], bf16, kind="ExternalInput")
    out_e = nc.dram_tensor("out", [TOK, C], bf16, kind="ExternalOutput")

    from contextlib import ExitStack

    with tile.TileContext(nc) as tc, ExitStack() as ctx:
        const = ctx.enter_context(tc.tile_pool(name="const", bufs=1))
        qkvp = ctx.enter_context(tc.tile_pool(name="qkv", bufs=1))
        psum = ctx.enter_context(tc.tile_pool(name="ps", bufs=2, space="PSUM"))
        xtp = ctx.enter_context(tc.tile_pool(name="xt", bufs=5))
        w1p = ctx.enter_context(tc.tile_pool(name="w1", bufs=1))
        rtp = ctx.enter_context(tc.tile_pool(name="rt", bufs=2))
        stag = ctx.enter_context(tc.tile_pool(name="stg", bufs=3))
        exp_p = ctx.enter_context(tc.tile_pool(name="exp", bufs=6))
        recp = ctx.enter_context(tc.tile_pool(name="rec", bufs=2))
        outp = ctx.enter_context(tc.tile_pool(name="outs", bufs=3))

        # ---- DMA emission order: get the first q-proj matmul running as
        # early as possible (wq/xt interleaved per k-tile), then the tables
        # needed by the first rope, then the rest in consumption order.
        xt_tiles = {}

        def load_xt_chunk(b, c):
            """One [C, NCH] token-chunk of x, one fully-contiguous DMA issue
            (the sync engine serializes dma_start issue at ~650ns each, so
            instruction count matters as much as transfer granularity)."""
            t = xtp.tile([128, KT, NCH], bf16, tag="xt")
            xt_tiles[(b, c)] = t
            nc.sync.dma_start(t[:], xt_e.ap()[:, b * QCH + c, :, :])

        wq_sb = w1p.tile([128, KT, NQH * D], bf16, tag="wq")
        nc.sync.dma_start(wq_sb[:], wq_e.ap())
        load_xt_chunk(0, 0)
        cos_sb = const.tile([D, T], bf16, tag="cos")
        nc.sync.dma_start(cos_sb[:], cos_e.ap())
        sin_sb = const.tile([D, T], bf16, tag="sin")
        nc.sync.dma_start(sin_sb[:], sin_e.ap())
        tri_sb = const.tile([D, D], bf16, tag="tri")
        nc.sync.dma_start(tri_sb[:], tri_e.ap())
        ones_sb = const.tile([128, 128], bf16, tag="ones")
        nc.vector.memset(ones_sb[:], 1.0)
        wk_sb = w1p.tile([128, KT, D], bf16, tag="wk")
        nc.sync.dma_start(wk_sb[:], wk_e.ap())
        wv_sb = w1p.tile([128, KT, D], bf16, tag="wv")
        nc.sync.dma_start(wv_sb[:], wv_e.ap())
        wp_sb = const.tile([128, NQH, C], bf16, tag="wp")
        nc.sync.dma_start(wp_sb[:], wp_e.ap())
        for tc_ in range(1, QCH):
            load_xt_chunk(0, tc_)

        # persistent per-batch-pair tensors
        qT = qkvp.tile([D, NQH, TOK], bf16, tag="qT")    # rope'd, pre-scaled
        kT = qkvp.tile([D, TOK], bf16, tag="kT")         # rope'd
        vv = qkvp.tile([128, B * KB, D], bf16, tag="vv")  # token-major
        yT = qkvp.tile([D, NQH, TOK], bf16, tag="yT")    # attn out, normalized

        def rope_out(dst, src_ps, cos_ap, sin_ap):
            """dst(bf16) = src * cos + rotate_half(src) * sin_signed.

            src is staged PSUM->SBUF bf16 on the scalar engine so all DVE
            ops below run in the 2x/4x (all-bf16, all-SBUF) perf modes.
            tensor_tensor operands must share a start partition, so the
            rotate-half is done with two (partition-shift-capable) copies."""
            qbf = stag.tile([128, NCH], bf16, tag="qbf")
            nc.scalar.copy(qbf[:], src_ps[:])
            rt = rtp.tile([128, NCH], bf16, tag="rt")
            nc.vector.tensor_copy(rt[0:64, :], qbf[64:128, :])
            nc.vector.tensor_copy(rt[64:128, :], qbf[0:64, :])
            m1 = rtp.tile([128, NCH], bf16, tag="m1")
            nc.vector.tensor_mul(m1[:], qbf[:], cos_ap)
            nc.vector.tensor_mul(rt[:], rt[:], sin_ap)
            nc.vector.tensor_tensor(out=dst, in0=m1[:], in1=rt[:], op=OP.add)

        def emit_proj_qt(b, qt, alt):
            """One out-projection unit: 128 output rows x full C."""
            tok0 = b * T
            osb = outp.tile([128, C], bf16, tag="osb")
            for fc in range(C // NCH):
                ops = psum.tile([128, NCH], f32, tag="mm")
                for kd in range(NQH):
                    nc.tensor.matmul(
                        ops[:],
                        yT[:, kd, tok0 + qt * 128: tok0 + (qt + 1) * 128],
                        wp_sb[:, kd, ts(fc, NCH)],
                        start=(kd == 0), stop=(kd == NQH - 1))
                if fc % 2 == alt:
                    nc.scalar.copy(osb[:, ts(fc, NCH)], ops[:])
                else:
                    nc.vector.tensor_copy(osb[:, ts(fc, NCH)], ops[:])
            nc.sync.dma_start(
                out_e.ap()[tok0 + qt * 128: tok0 + (qt + 1) * 128, :],
                osb[:])

        pending = []
        for b in range(B):
            tok0 = b * T
            # Per chunk: QKV projection + RoPE, then immediately the causal
            # attention for that q-chunk (its k/v prefix just completed) with
            # the previous chunk's out-projection interleaved at k-tile
            # granularity. This keeps the PE fed while xt DMA streams in and
            # never leaves a >3.4us idle window (HAM stays warm).
            for qc in range(QCH):
                xt_sb = xt_tiles[(b, qc)]
                cs = cos_sb[:, ts(qc, NCH)]
                sn = sin_sb[:, ts(qc, NCH)]
                for h in range(NQH):
                    ps = psum.tile([128, NCH], f32, tag="mm")
                    for kt in range(KT):
                        nc.tensor.matmul(ps[:],
                                         wq_sb[:, kt, h * D:(h + 1) * D],
                                         xt_sb[:, kt, :],
                                         start=(kt == 0), stop=(kt == KT - 1))
                    rope_out(qT[:, h, tok0 + qc * NCH: tok0 + (qc + 1) * NCH],
                             ps, cs, sn)
                ps = psum.tile([128, NCH], f32, tag="mm")
                for kt in range(KT):
                    nc.tensor.matmul(ps[:], wk_sb[:, kt, :],
                                     xt_sb[:, kt, :],
                                     start=(kt == 0), stop=(kt == KT - 1))
                rope_out(kT[:, tok0 + qc * NCH: tok0 + (qc + 1) * NCH],
                         ps, cs, sn)
                for ti in range(4):
                    ps = psum.tile([128, D], f32, tag="mm")
                    for kt in range(KT):
                        nc.tensor.matmul(ps[:],
                                         xt_sb[:, kt, ti * 128:(ti + 1) * 128],
                                         wv_sb[:, kt, :],
                                         start=(kt == 0), stop=(kt == KT - 1))
                    nc.scalar.copy(vv[:, b * KB + 4 * qc + ti, :], ps[:])
                if b == 0:
                    # prefetch batch 1's chunk: its WAR on this buffer just
                    # cleared, so it streams ~50us ahead of its use.
                    load_xt_chunk(1, qc)

                n_kt = 4 * qc + 4
                stride = max(1, (2 * n_kt) // 5)
                tcount = 0
                for h in range(NQH):
                    yps = psum.tile([128, NCH], f32, tag="y", bufs=2)
                    sps = psum.tile([128, NCH], f32, tag="s", bufs=2)
                    for kti in range(n_kt):
                        dq = kti - 4 * qc
                        c0 = dq * 128 if dq > 0 else 0  # masked cols skipped
                        q_sl = qT[:, h, tok0 + qc * NCH + c0:
                                  tok0 + (qc + 1) * NCH]
                        sc = psum.tile([128, NCH], f32, tag="sc", bufs=2)
                        nc.tensor.matmul(sc[:, c0:],
                                         kT[:, tok0 + kti * 128: tok0 + (kti + 1) * 128],
                                         q_sl, start=True, stop=True)
                        ex = exp_p.tile([128, NCH], bf16, tag="ex")
                        nc.scalar.activation(ex[:, c0:], sc[:, c0:], AF.Exp)
                        if dq >= 0:
                            nc.vector.tensor_mul(ex[:, ts(dq, 128)],
                                                 ex[:, ts(dq, 128)], tri_sb[:])
                        st = (kti == 0)
                        sp = (kti == n_kt - 1)
                        nc.tensor.matmul(yps[:, c0:], vv[:, b * KB + kti, :],
                                         ex[:, c0:], start=st, stop=sp)
                        nc.tensor.matmul(sps[:, c0:], ones_sb[:], ex[:, c0:],
                                         start=st, stop=sp)
                        tcount += 1
                        if tcount % stride == 0 and pending:
                            pending.pop(0)()
                    rec = recp.tile([128, NCH], f32, tag="rec")
                    nc.vector.reciprocal_approx_fast(out=rec[:], in_=sps[:])
                    nc.vector.tensor_mul(
                        yT[:, h, tok0 + qc * NCH: tok0 + (qc + 1) * NCH],
                        yps[:], rec[:])
                while pending:
                    pending.pop(0)()
                pending = [
                    (lambda b_=b, qt_=qt, alt_=qt % 2:
                     emit_proj_qt(b_, qt_, alt_))
                    for qt in range(4 * qc, 4 * qc + 4)]
        while pending:
            pending.pop(0)()

    nc.compile()
    return nc


def _get_nc():
    if "nc" not in _COMPILED:
        _COMPILED["nc"] = _build_nc()
    return _COMPILED["nc"]


def _tile_ct(w):
    """[C_contract, N] -> [128, C//128, N] (partition-major k-tiles)."""
    ct, n = w.shape
    return np.ascontiguousarray(
        w.reshape(ct // 128, 128, n).transpose(1, 0, 2)).astype(BF16)


def _stage_inputs(x, Wq, Wkv, Wproj):
    # x: [B,T,C] -> channel-major, chunk-major tiles [128, B*QCH, KT, NCH]
    xt = x.reshape(B * QCH, NCH, KT, 128).transpose(3, 0, 2, 1)
    xt = np.ascontiguousarray(xt).astype(BF16)
    cos, sin_s = _rope_tables()
    cos = cos.astype(BF16)
    sin_s = sin_s.astype(BF16)
    kk, qq = np.meshgrid(np.arange(D), np.arange(D), indexing="ij")
    tri = (kk <= qq).astype(BF16)                               # [k, q]

    in_maps = []
    for c in range(N_CORES):
        g = c // 2
        wq = _tile_ct(Wq[2 * c * D:(2 * c + 2) * D, :].T * SCALE)
        wk = _tile_ct(Wkv[g * D:(g + 1) * D, :].T)
        wv = _tile_ct(Wkv[4 * D + g * D: 4 * D + (g + 1) * D, :].T)
        wp = np.ascontiguousarray(
            Wproj[:, 2 * c * D:(2 * c + 2) * D].T
            .reshape(NQH, 128, C).transpose(1, 0, 2)).astype(BF16)
        in_maps.append({
            "xt": xt, "wq": wq, "wk": wk, "wv": wv, "wp": wp,
            "cos": cos, "sin": sin_s, "tri": tri,
        })
    return in_maps


def run(x, Wq, Wkv, Wproj, trace=False):
    from concourse.bass_utils import run_bass_kernel_spmd

    nc = _get_nc()
    in_maps = _stage_inputs(x, Wq, Wkv, Wproj)
    res = run_bass_kernel_spmd(nc, in_maps, core_ids=list(range(N_CORES)),
                               trace=trace)
    acc = np.zeros((TOK, C), np.float32)
    for c in range(N_CORES):
        acc += np.asarray(res.results[c]["out"], np.float32)
    out = acc.reshape(B, T, C)
    return (out, res) if trace else (out, None)


def kernel(x, Wq, Wkv, Wproj):
    out, _ = run(np.asarray(x, np.float32), np.asarray(Wq, np.float32),
                 np.asarray(Wkv, np.float32), np.asarray(Wproj, np.float32))
    return out


# revision 14
# speedup vs baseline: 1.2990x; 1.0742x over previous
"""Causal GQA self-attention (B=2, T=2048, D=2048, 16 q-heads / 4 kv-heads,
head_dim=128, full-dim RoPE) on 8 Trainium2 NeuronCores.

Strategy: tensor-parallel over heads. Core c owns q-heads {2c, 2c+1} and
kv-head c//2. Wq/Wkv output dims and Wproj input dims are sharded 8-ways on
the host; each core computes a full-width partial of the output projection
(bf16) and the host sums the 8 partials in f32.

On-chip layout: x is staged transposed (channel-major [C, B*T]) so the
QKV projections need no on-chip transpose; attention is computed "k-major"
(scores transposed, [k_pos, q_pos]) so the P@V contraction needs no
transpose either. Softmax runs without max-subtraction (scores are ~N(0,1);
exp never overflows) and the denominator comes from an all-ones stationary
matmul which also broadcasts the per-column sums across all partitions.

Scheduling notes (v2):
- RoPE runs in bf16: the projection PSUM is staged to SBUF via an ACT copy
  (scalar engine is idle in phase 1) so every DVE op is all-bf16/SBUF and
  hits the 2x perf mode. The rotate-half sign is folded into the sin table.
- Output partials are written bf16 (halves the 33MB write DMA).
- The out-projection for chunk qc-1 is interleaved into chunk qc's
  attention stream at k-tile granularity so its PSUM->SBUF copies never
  form a burst ahead of the next exp in the ACT queue.
- PSUM tags: mm(2) sc(2) y(2) s(2) = 8 banks, no cross-phase sharing.
- DMA emission order: wq/xt-chunk0 interleaved per k-tile, then cos/sin,
  wk/wv, remaining chunks, tri, wp last; batch-1 xt is emitted right after
  batch-0's QKV loop so it streams during batch-0 attention.
"""

import math
import os
import sys

for _p in ("/opt/trn_rl_repo", "/root/.axon_site/_ro/trn_rl_repo"):
    if os.path.isdir(_p) and _p not in sys.path:
        sys.path.insert(0, _p)

import ml_dtypes
import numpy as np

BF16 = ml_dtypes.bfloat16

B = 2
T = 2048
C = 2048
D = 128          # head dim
NQH = 2          # q heads per core
TOK = B * T      # 4096
KT = C // 128    # 16 contraction tiles
NCH = 512        # matmul moving-dim chunk
QCH = T // NCH   # 4 q chunks per batch
KB = T // 128    # 16 k tiles per batch
N_CORES = 8
SCALE = 1.0 / math.sqrt(D)

_COMPILED = {}


def _rope_tables():
    dim = np.arange(D // 2, dtype=np.float64)
    freq = 10000.0 ** (dim / (D / 2))
    freq = np.concatenate([freq, freq])              # [128]
    pos = np.arange(T, dtype=np.float64)
    ang = pos[None, :] / freq[:, None]               # [128, T] channel-major
    cos = np.cos(ang)
    sin_s = np.sin(ang)
    sin_s[: D // 2] = -sin_s[: D // 2]               # rotate-half sign folded
    return cos, sin_s


def _build_nc(debug=False):
    import concourse.bass as bass  # noqa: F401
    import concourse.mybir as mybir
    import concourse.tile as tile
    from concourse import bacc
    from concourse.bass import ts

    f32 = mybir.dt.float32
    bf16 = mybir.dt.bfloat16
    AF = mybir.ActivationFunctionType
    OP = mybir.AluOpType

    nc = bacc.Bacc("TRN2", target_bir_lowering=False, debug=False,
                   num_devices=N_CORES)

    # all inputs are host-pre-tiled to the exact SBUF layout so every DMA
    # moves long contiguous rows (16KB/partition) at full HBM bandwidth
    xt_e = nc.dram_tensor("xt", [128, B * QCH, KT, NCH], bf16,
                          kind="ExternalInput")
    wq_e = nc.dram_tensor("wq", [128, KT, NQH * D], bf16, kind="ExternalInput")
    wk_e = nc.dram_tensor("wk", [128, KT, D], bf16, kind="ExternalInput")
    wv_e = nc.dram_tensor("wv", [128, KT, D], bf16, kind="ExternalInput")
    wp_e = nc.dram_tensor("wp", [128, NQH, C], bf16, kind="ExternalInput")
    cos_e = nc.dram_tensor("cos", [D, T], bf16, kind="ExternalInput")
    sin_e = nc.dram_tensor("sin", [D, T], bf16, kind="ExternalInput")
    tri_e = nc.dram_tensor("tri", [D, D], bf16, kind="ExternalInput")
    out_e = nc.dram_tensor("out", [TOK, C], bf16, kind="ExternalOutput")

    from contextlib import ExitStack

    with tile.TileContext(nc) as tc, ExitStack() as ctx:
        const = ctx.enter_context(tc.tile_pool(name="const", bufs=1))
        qkvp = ctx.enter_context(tc.tile_pool(name="qkv", bufs=1))
        psum = ctx.enter_context(tc.tile_pool(name="ps", bufs=2, space="PSUM"))
        xtp = ctx.enter_context(tc.tile_pool(name="xt", bufs=5))
        w1p = ctx.enter_context(tc.tile_pool(name="w1", bufs=1))
        rtp = ctx.enter_context(tc.tile_pool(name="rt", bufs=2))
        stag = ctx.enter_context(tc.tile_pool(name="stg", bufs=3))
        exp_p = ctx.enter_context(tc.tile_pool(name="exp", bufs=6))
        recp = ctx.enter_context(tc.tile_pool(name="rec", bufs=2))
        outp = ctx.enter_context(tc.tile_pool(name="outs", bufs=3))

        # ---- DMA emission order: get the first q-proj matmul running as
        # early as possible (wq/xt interleaved per k-tile), then the tables
        # needed by the first rope, then the rest in consumption order.
        xt_tiles = {}

        def load_xt_chunk(b, c):
            """One [C, NCH] token-chunk of x, one fully-contiguous DMA issue
            (the sync engine serializes dma_start issue at ~650ns each, so
            instruction count matters as much as transfer granularity)."""
            t = xtp.tile([128, KT, NCH], bf16, tag="xt")
            xt_tiles[(b, c)] = t
            nc.sync.dma_start(t[:], xt_e.ap()[:, b * QCH + c, :, :])

        # first-use tensors split across DMA queues (~200GB/s per queue) so
        # the first q-projection can start ~10us in instead of ~17us
        wq_sb = w1p.tile([128, KT, NQH * D], bf16, tag="wq")
        xt00 = xtp.tile([128, KT, NCH], bf16, tag="xt")
        xt_tiles[(0, 0)] = xt00
        nc.sync.dma_start(wq_sb[:, 0:KT // 2, :], wq_e.ap()[:, 0:KT // 2, :])
        nc.sync.dma_start(xt00[:, 0:4, :], xt_e.ap()[:, 0, 0:4, :])
        nc.sync.dma_start(wq_sb[:, KT // 2:, :], wq_e.ap()[:, KT // 2:, :])
        nc.sync.dma_start(xt00[:, 4:8, :], xt_e.ap()[:, 0, 4:8, :])
        nc.sync.dma_start(xt00[:, 8:12, :], xt_e.ap()[:, 0, 8:12, :])
        nc.sync.dma_start(xt00[:, 12:16, :], xt_e.ap()[:, 0, 12:16, :])
        cos_sb = const.tile([D, T], bf16, tag="cos")
        nc.sync.dma_start(cos_sb[:], cos_e.ap())
        sin_sb = const.tile([D, T], bf16, tag="sin")
        nc.sync.dma_start(sin_sb[:], sin_e.ap())
        tri_sb = const.tile([D, D], bf16, tag="tri")
        nc.sync.dma_start(tri_sb[:], tri_e.ap())
        ones_sb = const.tile([128, 128], bf16, tag="ones")
        nc.vector.memset(ones_sb[:], 1.0)
        wk_sb = w1p.tile([128, KT, D], bf16, tag="wk")
        nc.sync.dma_start(wk_sb[:], wk_e.ap())
        wv_sb = w1p.tile([128, KT, D], bf16, tag="wv")
        nc.sync.dma_start(wv_sb[:], wv_e.ap())
        wp_sb = const.tile([128, NQH, C], bf16, tag="wp")
        nc.sync.dma_start(wp_sb[:], wp_e.ap())
        for tc_ in range(1, QCH):
            load_xt_chunk(0, tc_)

        # persistent per-batch-pair tensors
        qT = qkvp.tile([D, NQH, TOK], bf16, tag="qT")    # rope'd, pre-scaled
        kT = qkvp.tile([D, TOK], bf16, tag="kT")         # rope'd
        vv = qkvp.tile([128, B * KB, D], bf16, tag="vv")  # token-major
        yT = qkvp.tile([D, NQH, TOK], bf16, tag="yT")    # attn out, normalized

        def rope_out(dst, src_ps, cos_ap, sin_ap):
            """dst(bf16) = src * cos + rotate_half(src) * sin_signed.

            src is staged PSUM->SBUF bf16 on the scalar engine so all DVE
            ops below run in the 2x/4x (all-bf16, all-SBUF) perf modes.
            tensor_tensor operands must share a start partition, so the
            rotate-half is done with two (partition-shift-capable) copies."""
            qbf = stag.tile([128, NCH], bf16, tag="qbf")
            nc.scalar.copy(qbf[:], src_ps[:])
            rt = rtp.tile([128, NCH], bf16, tag="rt")
            nc.vector.tensor_copy(rt[0:64, :], qbf[64:128, :])
            nc.vector.tensor_copy(rt[64:128, :], qbf[0:64, :])
            m1 = rtp.tile([128, NCH], bf16, tag="m1")
            nc.vector.tensor_mul(m1[:], qbf[:], cos_ap)
            nc.vector.tensor_mul(rt[:], rt[:], sin_ap)
            nc.vector.tensor_tensor(out=dst, in0=m1[:], in1=rt[:], op=OP.add)

        def emit_proj_qt(b, qt, alt):
            """One out-projection unit: 128 output rows x full C."""
            tok0 = b * T
            osb = outp.tile([128, C], bf16, tag="osb")
            for fc in range(C // NCH):
                ops = psum.tile([128, NCH], f32, tag="mm")
                for kd in range(NQH):
                    nc.tensor.matmul(
                        ops[:],
                        yT[:, kd, tok0 + qt * 128: tok0 + (qt + 1) * 128],
                        wp_sb[:, kd, ts(fc, NCH)],
                        start=(kd == 0), stop=(kd == NQH - 1))
                if fc % 2 == alt:
                    nc.scalar.copy(osb[:, ts(fc, NCH)], ops[:])
                else:
                    nc.vector.tensor_copy(osb[:, ts(fc, NCH)], ops[:])
            nc.sync.dma_start(
                out_e.ap()[tok0 + qt * 128: tok0 + (qt + 1) * 128, :],
                osb[:])

        pending = []
        for b in range(B):
            tok0 = b * T
            # Per chunk: QKV projection + RoPE, then immediately the causal
            # attention for that q-chunk (its k/v prefix just completed) with
            # the previous chunk's out-projection interleaved at k-tile
            # granularity. This keeps the PE fed while xt DMA streams in and
            # never leaves a >3.4us idle window (HAM stays warm).
            for qc in range(QCH):
                xt_sb = xt_tiles[(b, qc)]
                cs = cos_sb[:, ts(qc, NCH)]
                sn = sin_sb[:, ts(qc, NCH)]
                for h in range(NQH):
                    ps = psum.tile([128, NCH], f32, tag="mm")
                    for kt in range(KT):
                        nc.tensor.matmul(ps[:],
                                         wq_sb[:, kt, h * D:(h + 1) * D],
                                         xt_sb[:, kt, :],
                                         start=(kt == 0), stop=(kt == KT - 1))
                    rope_out(qT[:, h, tok0 + qc * NCH: tok0 + (qc + 1) * NCH],
                             ps, cs, sn)
                ps = psum.tile([128, NCH], f32, tag="mm")
                for kt in range(KT):
                    nc.tensor.matmul(ps[:], wk_sb[:, kt, :],
                                     xt_sb[:, kt, :],
                                     start=(kt == 0), stop=(kt == KT - 1))
                rope_out(kT[:, tok0 + qc * NCH: tok0 + (qc + 1) * NCH],
                         ps, cs, sn)
                for ti in range(4):
                    ps = psum.tile([128, D], f32, tag="mm")
                    for kt in range(KT):
                        nc.tensor.matmul(ps[:],
                                         xt_sb[:, kt, ti * 128:(ti + 1) * 128],
                                         wv_sb[:, kt, :],
                                         start=(kt == 0), stop=(kt == KT - 1))
                    nc.scalar.copy(vv[:, b * KB + 4 * qc + ti, :], ps[:])
                if b == 0:
                    # prefetch batch 1's chunk: its WAR on this buffer just
                    # cleared, so it streams ~50us ahead of its use.
                    load_xt_chunk(1, qc)

                n_kt = 4 * qc + 4
                stride = max(1, (2 * n_kt) // 5)
                tcount = 0
                for h in range(NQH):
                    yps = psum.tile([128, NCH], f32, tag="y", bufs=2)
                    exsum = recp.tile([128, NCH], bf16, tag="exsum", bufs=2)
                    for kti in range(n_kt):
                        dq = kti - 4 * qc
                        c0 = dq * 128 if dq > 0 else 0  # masked cols skipped
                        q_sl = qT[:, h, tok0 + qc * NCH + c0:
                                  tok0 + (qc + 1) * NCH]
                        sc = psum.tile([128, NCH], f32, tag="sc", bufs=3)
                        nc.tensor.matmul(sc[:, c0:],
                                         kT[:, tok0 + kti * 128: tok0 + (kti + 1) * 128],
                                         q_sl, start=True, stop=True)
                        ex = exp_p.tile([128, NCH], bf16, tag="ex")
                        nc.scalar.activation(ex[:, c0:], sc[:, c0:], AF.Exp)
                        if dq >= 0:
                            nc.vector.tensor_mul(ex[:, ts(dq, 128)],
                                                 ex[:, ts(dq, 128)], tri_sb[:])
                        st = (kti == 0)
                        sp = (kti == n_kt - 1)
                        nc.tensor.matmul(yps[:, c0:], vv[:, b * KB + kti, :],
                                         ex[:, c0:], start=st, stop=sp)
                        # denominator partials accumulate on DVE (2x bf16
                        # mode); the partition sum + broadcast is a single
                        # ones-matmul on the final exsum instead of one per
                        # k-tile (saves ~25us of PE).
                        if kti == 0:
                            nc.vector.tensor_copy(exsum[:], ex[:])
                        else:
                            nc.vector.tensor_tensor(
                                out=exsum[:, c0:], in0=exsum[:, c0:],
                                in1=ex[:, c0:], op=OP.add)
                        tcount += 1
                        if tcount % stride == 0 and pending:
                            pending.pop(0)()
                    sps = psum.tile([128, NCH], f32, tag="s", bufs=1)
                    nc.tensor.matmul(sps[:], ones_sb[:], exsum[:],
                                     start=True, stop=True)
                    rec = recp.tile([128, NCH], f32, tag="rec")
                    nc.vector.reciprocal_approx_fast(out=rec[:], in_=sps[:])
                    nc.vector.tensor_mul(
                        yT[:, h, tok0 + qc * NCH: tok0 + (qc + 1) * NCH],
                        yps[:], rec[:])
                while pending:
                    pending.pop(0)()
                pending = [
                    (lambda b_=b, qt_=qt, alt_=qt % 2:
                     emit_proj_qt(b_, qt_, alt_))
                    for qt in range(4 * qc, 4 * qc + 4)]
        while pending:
            pending.pop(0)()

    nc.compile()
    return nc


def _get_nc():
    if "nc" not in _COMPILED:
        _COMPILED["nc"] = _build_nc()
    return _COMPILED["nc"]


def _tile_ct(w):
    """[C_contract, N] -> [128, C//128, N] (partition-major k-tiles)."""
    ct, n = w.shape
    return np.ascontiguousarray(
        w.reshape(ct // 128, 128, n).transpose(1, 0, 2)).astype(BF16)


def _stage_inputs(x, Wq, Wkv, Wproj):
    # x: [B,T,C] -> channel-major, chunk-major tiles [128, B*QCH, KT, NCH]
    xt = x.reshape(B * QCH, NCH, KT, 128).transpose(3, 0, 2, 1)
    xt = np.ascontiguousarray(xt).astype(BF16)
    cos, sin_s = _rope_tables()
    cos = cos.astype(BF16)
    sin_s = sin_s.astype(BF16)
    kk, qq = np.meshgrid(np.arange(D), np.arange(D), indexing="ij")
    tri = (kk <= qq).astype(BF16)                               # [k, q]

    in_maps = []
    for c in range(N_CORES):
        g = c // 2
        wq = _tile_ct(Wq[2 * c * D:(2 * c + 2) * D, :].T * SCALE)
        wk = _tile_ct(Wkv[g * D:(g + 1) * D, :].T)
        wv = _tile_ct(Wkv[4 * D + g * D: 4 * D + (g + 1) * D, :].T)
        wp = np.ascontiguousarray(
            Wproj[:, 2 * c * D:(2 * c + 2) * D].T
            .reshape(NQH, 128, C).transpose(1, 0, 2)).astype(BF16)
        in_maps.append({
            "xt": xt, "wq": wq, "wk": wk, "wv": wv, "wp": wp,
            "cos": cos, "sin": sin_s, "tri": tri,
        })
    return in_maps


def run(x, Wq, Wkv, Wproj, trace=False):
    from concourse.bass_utils import run_bass_kernel_spmd

    nc = _get_nc()
    in_maps = _stage_inputs(x, Wq, Wkv, Wproj)
    res = run_bass_kernel_spmd(nc, in_maps, core_ids=list(range(N_CORES)),
                               trace=trace)
    acc = np.zeros((TOK, C), np.float32)
    for c in range(N_CORES):
        acc += np.asarray(res.results[c]["out"], np.float32)
    out = acc.reshape(B, T, C)
    return (out, res) if trace else (out, None)


def kernel(x, Wq, Wkv, Wproj):
    out, _ = run(np.asarray(x, np.float32), np.asarray(Wq, np.float32),
                 np.asarray(Wkv, np.float32), np.asarray(Wproj, np.float32))
    return out
